# revision 3
# baseline (speedup 1.0000x reference)
"""Trainium2 Bass kernel for nn_DQN_34136400069239 (DeepSets-style pooling).

Math (reference):
    h1  = relu(x @ pw1 + pb1)          [N, H]
    h2  = relu(h1 @ pw2 + pb2)         [N, H]
    phi = h2 @ pw3 + pb3               [N, F]
    fp  = sum(phi, axis=0)             [F]
    ... tiny rho MLP + concat(x_static) + tiny 3-layer MLP -> [OUT]

The third phi layer is linear, so fp = (sum_n h2[n]) @ pw3 + N * pb3 and the
device only computes S = sum_n relu(h2[n]) in R^H.  Data-parallel over rows:
8 cores x 50000 rows, host sums the 8 partial S vectors and runs the tail.

Device design (mode "v3*"), per 1000-row pair of 500-row blocks:
  - PSUM is laid out pair-level and half-major: ps1_h0/ps1_h1 and
    ps2_h0/ps2_h1 are [128, 2(block), 512] tiles.  Each vector-engine op
    then covers ONE h-half of TWO blocks, so its per-partition bias vector
    is uniform and accum_out keeps per-channel sums:
      DVE: h1 = max(ps1_h + b1_h, 0)   tensor_scalar(add, max), FD=1000
      ACT: relu(ps2_h + b2_h) + row-sum accum_out, FD=1000
    Those two engines are the roofline (~1167 / ~977 ns per 500-row block);
    biases ride along for free as exact-f32 per-partition operands.
  - mode "v3"    : layer 2 = 4 f16 matmuls (K=128) per block, PE ~3000 c/blk
  - mode "v3fp8" : layer 2 = 2 fp8e4m3 DoubleRow matmuls (K_eff=256) per
                   block, PE ~2100 c/blk; W2 quantized with error diffusion
                   down the contraction axis (plain fp8 rounding fails the
                   2e-2 gate at 2.3e-2; diffusion passes at ~3.5e-3).
"""

import os

import numpy as np

# Problem constants (hardcoded; kernel.py must be self-contained).
N = 400000
IN, H, F, S_STATIC, OUT = 64, 256, 128, 16, 5
N_CORES = 8
R = N // N_CORES  # rows per core = 50000
BLK = 500  # matmul moving free dim
NBLK = R // BLK  # 100
NPAIR = NBLK // 2  # 50

MODE = os.environ.get("DQN_MODE", "v3fp8")

_prog_cache: dict = {}


def _build(mode: str, iters: int = 1):
    import concourse.mybir as mybir
    import concourse.tile as tile
    from concourse import bacc
    from contextlib import ExitStack

    dt = mybir.dt
    f32 = dt.float32
    f16 = dt.float16
    fp8 = mode == "v3fp8"
    h1_dt = dt.float8e4 if fp8 else f16

    nc = bacc.Bacc(
        "TRN2",
        target_bir_lowering=False,
        debug=False,
        enable_asserts=False,
        num_devices=1,
    )

    d_xt = nc.dram_tensor("d_xt", [IN, R], f16, kind="ExternalInput").ap()
    d_w1 = nc.dram_tensor("d_w1", [IN, H], f16, kind="ExternalInput").ap()
    if fp8:
        # packed [k, pair, m]: W2p[k, i, m] = W2q[128*i + k, m]
        d_w2 = nc.dram_tensor("d_w2", [128, 2, H], dt.float8e4, kind="ExternalInput").ap()
    else:
        d_w2 = nc.dram_tensor("d_w2", [H, H], f16, kind="ExternalInput").ap()
    # f32 per-partition biases: cols = [b1_h0, b1_h1, b2_h0, b2_h1]
    d_b = nc.dram_tensor("d_b", [128, 4], f32, kind="ExternalInput").ap()
    d_s = nc.dram_tensor("d_s", [128, 2], f32, kind="ExternalOutput").ap()

    Relu = mybir.ActivationFunctionType.Relu
    Alu = mybir.AluOpType
    X = mybir.AxisListType.X

    with tile.TileContext(nc) as tc, ExitStack() as ctx:
        cpool = ctx.enter_context(tc.tile_pool(name="cpool", bufs=1))
        xpool = ctx.enter_context(tc.tile_pool(name="xpool", bufs=3))
        hpool = ctx.enter_context(tc.tile_pool(name="hpool", bufs=2))
        spool = ctx.enter_context(tc.tile_pool(name="spool", bufs=1))
        ps1p = ctx.enter_context(tc.tile_pool(name="ps1p", bufs=1, space="PSUM"))
        ps2p = ctx.enter_context(tc.tile_pool(name="ps2p", bufs=1, space="PSUM"))

        # Constants resident in SBUF.
        w1_sb = cpool.tile([IN, H], f16, name="w1_sb")
        nc.sync.dma_start(w1_sb[:], d_w1)
        if fp8:
            w2p_sb = cpool.tile([128, 2, H], dt.float8e4, name="w2p_sb")
            nc.sync.dma_start(w2p_sb[:], d_w2)
        else:
            w2_sb = []
            for k in range(2):
                t = cpool.tile([128, H], f16, name=f"w2_sb{k}")
                nc.sync.dma_start(t[:], d_w2[k * 128 : (k + 1) * 128, :])
                w2_sb.append(t)
        bv = cpool.tile([128, 4], f32, name="bv")
        nc.sync.dma_start(bv[:], d_b)

        # Per-pair accumulated row-sums of relu(h2), one column per pair.
        acc = [cpool.tile([128, NPAIR], f32, name=f"acc{m}") for m in range(2)]

        for pair in [p for _ in range(iters) for p in range(NPAIR)]:
            xt = xpool.tile([IN, 2 * BLK], f16, name="xt", tag="xt")
            nc.sync.dma_start(xt[:], d_xt[:, pair * 2 * BLK : (pair + 1) * 2 * BLK])

            ps1 = [
                ps1p.tile([128, 2, 512], f32, name=f"ps1_{m}", tag=f"ps1_{m}")
                for m in range(2)
            ]
            ps2 = [
                ps2p.tile([128, 2, 512], f32, name=f"ps2_{m}", tag=f"ps2_{m}")
                for m in range(2)
            ]

            # Layer 1: 4 K=64 matmuls into half-major pair psum.
            for j in range(2):
                xr = xt[:, j * BLK : (j + 1) * BLK]
                for m in range(2):
                    nc.tensor.matmul(
                        ps1[m][:, j, 0:BLK],
                        w1_sb[:, m * 128 : (m + 1) * 128],
                        xr,
                        start=True,
                        stop=True,
                    )

            # h1 = relu(ps1 + b1): one DVE op per half (uniform bias vector).
            h1 = hpool.tile([128, 2, 2, 512], h1_dt, name="h1", tag="h1")
            for m in range(2):
                nc.vector.tensor_scalar(
                    h1[:, m, :, 0:BLK],
                    ps1[m][:, :, 0:BLK],
                    bv[:, m : m + 1],
                    0.0,
                    op0=Alu.add,
                    op1=Alu.max,
                )

            # Layer 2 into pair-level psum.
            for j in range(2):
                if fp8:
                    for m in range(2):
                        nc.tensor.matmul(
                            ps2[m][:, j, 0:BLK],
                            w2p_sb[:, :, m * 128 : (m + 1) * 128],
                            h1[:, :, j, 0:BLK],
                            start=True,
                            stop=True,
                            perf_mode=mybir.MatmulPerfMode.DoubleRow,
                        )
                else:
                    for m in range(2):
                        for k in range(2):
                            nc.tensor.matmul(
                                ps2[m][:, j, 0:BLK],
                                w2_sb[k][:, m * 128 : (m + 1) * 128],
                                h1[:, k, j, 0:BLK],
                                start=(k == 0),
                                stop=(k == 1),
                            )

            # relu(ps2 + b2) with fused row-sum; channels preserved because
            # each op spans one half of both blocks.
            for m in range(2):
                scr = spool.tile([128, 2, 512], f16, name=f"scr{m}", tag=f"scr{m}")
                nc.scalar.activation(
                    scr[:, :, 0:BLK],
                    ps2[m][:, :, 0:BLK],
                    Relu,
                    bias=bv[:, 2 + m : 3 + m],
                    accum_out=acc[m][:, pair : pair + 1],
                )

        s_sb = cpool.tile([128, 2], f32, name="s_sb")
        for m in range(2):
            nc.vector.reduce_sum(s_sb[:, m : m + 1], acc[m][:], axis=X)
        nc.sync.dma_start(d_s, s_sb[:])

    nc.compile()
    return nc


def _diffuse_quant(W: np.ndarray, qdt) -> np.ndarray:
    """Error-diffusion quantization down the contraction axis: keeps
    per-column cumulative quantization error near zero so the (positive-mean)
    h1 stream doesn't see a systematic bias."""
    Wq = np.empty(W.shape, np.float32)
    carry = np.zeros(W.shape[1], np.float32)
    for k in range(W.shape[0]):
        t = W[k] + carry
        q = t.astype(qdt).astype(np.float32)
        carry = t - q
        Wq[k] = q
    return Wq


def _prep_in_maps(inputs: dict, mode: str):
    import ml_dtypes

    fp8 = mode == "v3fp8"
    x = np.asarray(inputs["x"], dtype=np.float32)
    pw1 = np.asarray(inputs["pw1"], dtype=np.float16)
    pb1 = np.asarray(inputs["pb1"], dtype=np.float32)
    pw2 = np.asarray(inputs["pw2"], dtype=np.float32)
    pb2 = np.asarray(inputs["pb2"], dtype=np.float32)

    if fp8:
        w2q = _diffuse_quant(pw2, ml_dtypes.float8_e4m3)
        w2 = np.ascontiguousarray(
            w2q.reshape(2, 128, H).transpose(1, 0, 2)
        ).astype(ml_dtypes.float8_e4m3)  # [k, pair, m]
    else:
        w2 = pw2.astype(np.float16)

    b = np.stack(
        [pb1[0:128], pb1[128:256], pb2[0:128], pb2[128:256]], axis=1
    ).astype(np.float32)  # [128, 4]

    in_maps = []
    for c in range(N_CORES):
        xt = np.ascontiguousarray(x[c * R : (c + 1) * R].T).astype(np.float16)
        in_maps.append({"d_xt": xt, "d_w1": pw1, "d_w2": w2, "d_b": b})
    return in_maps


def _host_tail(S: np.ndarray, inputs: dict) -> np.ndarray:
    f = np.float64

    def g(name):
        return np.asarray(inputs[name], dtype=f)

    phi_sum = S @ g("pw3") + N * g("pb3")
    r = np.maximum(phi_sum @ g("rw1") + g("rb1"), 0.0)
    r = np.maximum(r @ g("rw2") + g("rb2"), 0.0)
    r = r @ g("rw3") + g("rb3")
    v = np.concatenate([r, g("x_static")])
    v = np.maximum(v @ g("w1") + g("b1"), 0.0)
    v = np.maximum(v @ g("w2") + g("b2"), 0.0)
    return (v @ g("w3") + g("b3")).astype(np.float32)


def _run(inputs: dict, trace: bool = False, mode: str | None = None):
    from concourse.bass_utils import run_bass_kernel_spmd

    mode = mode or MODE
    nc = _prog_cache.get(mode)
    if nc is None:
        nc = _build(mode)
        _prog_cache[mode] = nc

    if trace:
        try:
            import antenv.axon_hooks  # noqa: F401
        except ImportError:
            trace = False

    in_maps = _prep_in_maps(inputs, mode)
    res = run_bass_kernel_spmd(
        nc,
        in_maps,
        core_ids=list(range(N_CORES)),
        trace=trace,
    )

    S = np.zeros(H, np.float64)
    for rmap in res.results:
        s = rmap["d_s"].astype(np.float64)  # [128, 2]; channel = m*128 + p
        S += s.T.reshape(H)
    out = _host_tail(S, inputs)
    return out, res


def kernel(**inputs) -> np.ndarray:
    out, _ = _run(inputs)
    return out
